# revision 31
# baseline (speedup 1.0000x reference)
"""GatedAttention TRN2 kernel — 8-core tensor-parallel (1 kv-head group per core).

Self-contained: host-side shard/layout prep + Bass/Tile kernel + gather.

Software-pipelined block structure: while DVE runs block B's RMS-Newton and
RoPE chains, the PE runs block B+1's QKV projection; block B-1's output
projection is deferred until after QKV(B+1) so the sigmoid/denominator chain
of B-1 is fully hidden.  Per-token scales are broadcast across partitions via
PE selector matmuls (no DMA broadcasts).
"""
import math
import os
import sys
import numpy as np
import ml_dtypes

BF16 = ml_dtypes.bfloat16

H, NH, KVH, HD = 2048, 32, 8, 64
G = NH // KVH          # 4 q heads per core
S = 2048
EPS = 1e-6
THETA = 1000000.0
SCALE = 1.0 / math.sqrt(HD)
NCORES = 8
HC = H // 128          # 16 h-chunks
NB = S // 512          # 4 si-blocks
NJ = S // 128          # 16 sj-chunks

_BUILT = {}
LAST_EXEC_NS = None


# ---------------------------------------------------------------- host prep
def _host_prep(hidden_states, Wq, Wk, Wv, Wo, g_q, g_k):
    x = np.ascontiguousarray(np.asarray(hidden_states, np.float32).reshape(S, H))
    Wq = np.asarray(Wq, np.float32)
    Wk = np.asarray(Wk, np.float32)
    Wv = np.asarray(Wv, np.float32)
    Wo = np.asarray(Wo, np.float32)
    g_q = np.asarray(g_q, np.float32)
    g_k = np.asarray(g_k, np.float32)

    xT = np.ascontiguousarray(x.T).astype(BF16)

    inv_freq = 1.0 / (THETA ** (np.arange(0, HD, 2, dtype=np.float32) / HD))
    pos = np.arange(S, dtype=np.float32)
    emb = np.concatenate([pos[:, None] * inv_freq[None, :]] * 2, axis=-1)  # [S,64]
    cos = np.cos(emb).T.astype(np.float32)   # [64, S]
    sin = np.sin(emb).T.astype(np.float32)
    sign = np.where(np.arange(HD) < HD // 2, -1.0, 1.0).astype(np.float32)[:, None]
    cosq = np.ascontiguousarray(cos * g_q[:, None]).astype(BF16)
    sinq = np.ascontiguousarray(sin * sign * np.roll(g_q, -32)[:, None]).astype(BF16)
    cosk = np.ascontiguousarray(cos * g_k[:, None]).astype(BF16)
    sink = np.ascontiguousarray(sin * sign * np.roll(g_k, -32)[:, None]).astype(BF16)

    def pmajor(a, nch):
        rows, wid = a.shape
        return np.ascontiguousarray(
            a.reshape(nch, 128, wid).transpose(1, 0, 2).reshape(128, nch * wid))

    in_maps = []
    for c in range(NCORES):
        Wq_g = Wq[:, c * (G * HD + G):(c + 1) * (G * HD + G)]
        gpad = np.zeros((H, 64), np.float32)
        for p in range(2):
            for hh in range(2):
                gpad[:, 32 * p + hh] = Wq_g[:, G * HD + 2 * p + hh]
        W_c = np.ascontiguousarray(np.concatenate(
            [Wq_g[:, :G * HD],
             Wk[:, c * HD:(c + 1) * HD],
             Wv[:, c * HD:(c + 1) * HD],
             gpad], axis=1))                                   # [H, 448]
        Wo_c = np.ascontiguousarray(Wo[c * G * HD:(c + 1) * G * HD, :])  # [256,H]
        eselb = np.kron(np.eye(2, dtype=np.float32), np.ones((1, 64), np.float32))
        xTcc = np.concatenate(
            [pmajor(np.ascontiguousarray(xT[:, q * 512:(q + 1) * 512]), HC)
             for q in range(4)], axis=1)                     # [128, 4*HC*512]
        in_maps.append({"xT": xTcc, "W": pmajor(W_c.astype(BF16), HC),
                        "Wo": pmajor(Wo_c.astype(BF16), 2),
                        "cosq": np.concatenate([cosq, cosq], 0),
                        "sinq": np.concatenate([sinq, sinq], 0),
                        "cosk": cosk, "sink": sink,
                        "eselb": eselb})
    return in_maps


# ---------------------------------------------------------------- bass build
def _build_nc():
    import concourse.bass as bass
    import concourse.mybir as mybir
    import concourse.tile as tile
    from concourse import bacc
    from concourse.masks import make_identity, make_upper_triangular

    dt = mybir.dt
    f32 = dt.float32
    bf16 = dt.bfloat16
    AF = mybir.ActivationFunctionType

    nc = bacc.Bacc("TRN2", target_bir_lowering=False, debug=False,
                   num_devices=NCORES)

    xT_d = nc.dram_tensor("xT", [128, HC * S], bf16, kind="ExternalInput")
    W_d = nc.dram_tensor("W", [128, HC * 448], bf16, kind="ExternalInput")
    Wo_d = nc.dram_tensor("Wo", [128, 2 * H], bf16, kind="ExternalInput")
    cosq_d = nc.dram_tensor("cosq", [128, S], bf16, kind="ExternalInput")
    sinq_d = nc.dram_tensor("sinq", [128, S], bf16, kind="ExternalInput")
    cosk_d = nc.dram_tensor("cosk", [HD, S], bf16, kind="ExternalInput")
    sink_d = nc.dram_tensor("sink", [HD, S], bf16, kind="ExternalInput")
    eselb_d = nc.dram_tensor("eselb", [2, 128], f32, kind="ExternalInput")
    out_d = nc.dram_tensor("out", [S, H], bf16, kind="ExternalOutput")

    import contextlib
    with tile.TileContext(nc) as tc, contextlib.ExitStack() as ctx:
        const = ctx.enter_context(tc.tile_pool(name="const", bufs=1))
        big = ctx.enter_context(tc.tile_pool(name="big", bufs=1))
        rawp = ctx.enter_context(tc.tile_pool(name="raw", bufs=2))
        tmpp = ctx.enter_context(tc.tile_pool(name="tmp", bufs=1))
        sqp = ctx.enter_context(tc.tile_pool(name="sq", bufs=2))
        expp = ctx.enter_context(tc.tile_pool(name="expp", bufs=4))
        outs = ctx.enter_context(tc.tile_pool(name="outs", bufs=2))
        smal = ctx.enter_context(tc.tile_pool(name="smal", bufs=2))
        bcp = ctx.enter_context(tc.tile_pool(name="bc", bufs=2))
        psum = ctx.enter_context(tc.tile_pool(name="ps", bufs=1, space="PSUM"))

        # ---------------- resident weights / tables / x
        # spread the prologue loads across several DGE engines so compute
        # can start as soon as W + the first xT chunks land
        W_sb = big.tile([128, HC, 448], bf16, tag="W")
        for h2 in range(2):
            eng = [nc.gpsimd, nc.scalar][h2]
            eng.dma_start(
                out=W_sb[:, h2 * 8:(h2 + 1) * 8, :],
                in_=W_d[:, h2 * 8 * 448:(h2 + 1) * 8 * 448].rearrange(
                    "p (hc c) -> p hc c", hc=8))
        xT_sb = big.tile([128, 4, HC, 512], bf16, tag="xT")
        for q4 in range(4):
            eng = [nc.gpsimd, nc.scalar, nc.gpsimd, nc.scalar][q4]
            eng.dma_start(
                out=xT_sb[:, q4, :, :],
                in_=xT_d[:, q4 * HC * 512:(q4 + 1) * HC * 512].rearrange(
                    "p (hc s) -> p hc s", hc=HC))
        eselb = const.tile([2, 128], f32, tag="eselb")
        nc.sync.dma_start(out=eselb, in_=eselb_d[:, :])

        cosq_sb = big.tile([128, S], bf16, tag="cosq")
        nc.scalar.dma_start(out=cosq_sb, in_=cosq_d[:, :])
        sinq_sb = big.tile([128, S], bf16, tag="sinq")
        nc.scalar.dma_start(out=sinq_sb, in_=sinq_d[:, :])
        cosk_sb = big.tile([64, S], bf16, tag="cosk")
        nc.scalar.dma_start(out=cosk_sb, in_=cosk_d[:, :])
        sink_sb = big.tile([64, S], bf16, tag="sink")
        nc.scalar.dma_start(out=sink_sb, in_=sink_d[:, :])
        Wo_sb = big.tile([128, 2, H], bf16, tag="Wo")
        nc.gpsimd.dma_start(out=Wo_sb, in_=Wo_d[:, :].rearrange(
            "p (cc h) -> p cc h", cc=2))

        # ---------------- constants
        id64 = const.tile([64, 64], bf16, tag="id64")
        make_identity(nc, id64)
        tri = const.tile([128, 128], bf16, tag="tri")
        make_upper_triangular(nc, tri, val=1.0, diag=True)
        ones = const.tile([128, 1], bf16, tag="ones")
        nc.vector.memset(ones, 1.0)
        esel = const.tile([128, 2], bf16, tag="esel")
        nc.vector.memset(esel, 0.0)
        nc.vector.memset(esel[0:64, 0:1], 1.0)
        nc.vector.memset(esel[64:128, 1:2], 1.0)
        SIGMA = 0.0430
        EXPBIT_SCALE = math.log(2.0) / (1 << 23)
        b_rsq = const.tile([128, 1], f32, tag="brsq")
        nc.vector.memset(b_rsq, 0.5 * math.log(2.0) * (127 + SIGMA + 6))
        u32 = dt.uint32

        # ---------------- persistent activations
        kk2 = big.tile([128, S], bf16, tag="kk2")
        v_sb = big.tile([128, NJ, 65], bf16, tag="v")
        nc.vector.memset(v_sb[:, :, 64:65], 1.0)
        rkT_sb = big.tile([128, NJ], f32, tag="rkT")

        st8 = {}   # per-block pipeline state

        def qkv_block(sib):
            """QKV+gate projection, PSUM->SBUF copies, exp(-gate)."""
            s = {}
            # three sequential single-bank passes (q0, q1, kv) + gate bank
            s["qr"] = [rawp.tile([128, 512], bf16, tag=f"qr{p}", name=f"qr{p}")
                       for p in range(2)]
            for p in range(2):
                ps_q = psum.tile([128, 512], f32, tag="qkv", bufs=1,
                                 name=f"psq{p}")
                for hc in range(HC):
                    nc.tensor.matmul(ps_q[:],
                                     W_sb[:, hc, p * 128:(p + 1) * 128],
                                     xT_sb[:, sib, hc, :],
                                     start=(hc == 0), stop=(hc == HC - 1))
                nc.scalar.copy(s["qr"][p], ps_q[:])
            ps_kv = psum.tile([128, 512], f32, tag="qkv", bufs=1, name="pskv")
            ps_g = psum.tile([64, 512], f32, tag="gate", bufs=1)
            for hc in range(HC):
                nc.tensor.matmul(ps_kv[:], W_sb[:, hc, 256:384],
                                 xT_sb[:, sib, hc, :],
                                 start=(hc == 0), stop=(hc == HC - 1))
                nc.tensor.matmul(ps_g[:], W_sb[:, hc, 384:448],
                                 xT_sb[:, sib, hc, :],
                                 start=(hc == 0), stop=(hc == HC - 1))
            s["kr"] = rawp.tile([64, 512], bf16, tag="kr", name="kr")
            s["vr"] = rawp.tile([64, 512], bf16, tag="vr", name="vr")
            nc.scalar.copy(s["kr"], ps_kv[0:64, :])
            nc.scalar.copy(s["vr"], ps_kv[64:128, :])
            s["sig"] = []
            for p in range(2):
                eg_t = smal.tile([2, 512], f32, tag="sig", bufs=4, name="eg")
                nc.scalar.activation(eg_t, ps_g[32 * p:32 * p + 2, :], AF.Exp,
                                     scale=-1.0)
                s["sig"].append(eg_t)
            st8[sib] = s

        def smalls(sib):
            """Selector matmuls + rsqrt seeds + V transposes (cheap PE/ACT)."""
            s = st8[sib]
            for j in range(4):
                J = sib * 4 + j
                ps_v = psum.tile([128, 64], bf16, tag="sc", bufs=2, name="psv")
                nc.tensor.transpose(ps_v[:], s["vr"][:, j * 128:(j + 1) * 128],
                                    id64)
                nc.scalar.copy(v_sb[:, J, 0:64], ps_v[:])
            ksq = sqp.tile([64, 512], bf16, tag="ksq")
            nc.vector.tensor_mul(ksq, s["kr"], s["kr"])
            ps_rk = psum.tile([128, 4], f32, tag="sc", bufs=2, name="psrk")
            for j in range(4):
                nc.tensor.matmul(ps_rk[:, j:j + 1],
                                 ksq[:, j * 128:(j + 1) * 128],
                                 ones[0:64, :], start=True, stop=True)
            s["ksum"] = smal.tile([128, 4], f32, tag="ksum", name="ksum")
            nc.vector.tensor_copy(s["ksum"], ps_rk[:])
            s["yk"] = smal.tile([128, 4], f32, tag="kB", name="yk")
            nc.scalar.activation(s["yk"], ps_rk[:].bitcast(u32), AF.Exp,
                                 bias=b_rsq, scale=-0.5 * EXPBIT_SCALE)
            s["qsum"], s["y0"] = [], []
            for p in range(2):
                sq = sqp.tile([128, 512], bf16, tag="sq", name=f"sq{p}")
                nc.vector.tensor_mul(sq, s["qr"][p], s["qr"][p])
                ps_rq = psum.tile([2, 512], f32, tag="sc", bufs=2,
                                  name=f"psrq{p}")
                nc.tensor.matmul(ps_rq[:], esel, sq, start=True, stop=True)
                qsum = smal.tile([2, 512], f32, tag="qsum", bufs=2, name="qsum")
                nc.vector.tensor_copy(qsum, ps_rq[:])
                y0 = smal.tile([2, 512], f32, tag="smB", bufs=4, name="y0")
                nc.scalar.activation(y0, ps_rq[:].bitcast(u32), AF.Exp,
                                     bias=b_rsq[0:2, :], scale=-0.5 * EXPBIT_SCALE)
                s["qsum"].append(qsum)
                s["y0"].append(y0)

        def newton_q(s, p, iters=1):
            y0 = s["y0"][p]
            for it in range(iters):
                tn = smal.tile([2, 512], f32, tag="smA", bufs=2, name="tn")
                nc.vector.tensor_mul(tn, s["qsum"][p], y0)
                nc.vector.tensor_mul(tn, tn, y0)
                nc.vector.tensor_scalar(tn, tn, -0.5 / HD, 1.5,
                                        mybir.AluOpType.mult,
                                        mybir.AluOpType.add)
                yn = smal.tile([2, 512], f32, tag="smB", bufs=4, name="yn")
                nc.vector.tensor_mul(yn, y0, tn)
                y0 = yn
            return y0

        def rope_q(s, sib, p):
            sp = slice(sib * 512, (sib + 1) * 512)
            ps_b = psum.tile([128, 512], f32, tag="sc", bufs=2,
                             name=f"psb{p}")
            nc.tensor.matmul(ps_b[:], eselb, s["rqt"][p][:],
                             start=True, stop=True)
            t1 = tmpp.tile([128, 512], bf16, tag="t1")
            nc.vector.tensor_mul(t1, s["qr"][p], cosq_sb[:, sp])
            qs = tmpp.tile([128, 512], bf16, tag="qs")
            for g in range(2):
                b = g * 64
                nc.vector.tensor_copy(qs[b:b + 32, :],
                                      s["qr"][p][b + 32:b + 64, :])
                nc.vector.tensor_copy(qs[b + 32:b + 64, :],
                                      s["qr"][p][b:b + 32, :])
            t2 = tmpp.tile([128, 512], bf16, tag="t2")
            nc.vector.tensor_mul(t2, qs, sinq_sb[:, sp])
            nc.vector.tensor_add(t2, t1, t2)
            nc.vector.tensor_mul(s["qf"][p], t2, ps_b[:])

        def chains_a(sib):
            """p0 rsqrt+RoPE, k rsqrt+RoPE -> qf0/kk2/rkT ready."""
            s = st8[sib]
            sp = slice(sib * 512, (sib + 1) * 512)
            s["qf"] = [rawp.tile([128, 512], bf16, tag=f"qf{p}",
                                 name=f"qf{p}", bufs=2) for p in range(2)]
            s["rqt"] = [None, None]
            s["rqt"][0] = newton_q(s, 0)
            # k rsqrt Newton on [128,4] (free dim 4 -> cheap)
            yk = s["yk"]
            for it in range(2):
                last = (it == 1)
                tk = smal.tile([128, 4], f32, tag="kA", name="tk")
                nc.vector.tensor_mul(tk, s["ksum"], yk)
                nc.vector.tensor_mul(tk, tk, yk)
                nc.vector.tensor_scalar(tk, tk,
                                        (-0.5 * SCALE / HD) if last else (-0.5 / HD),
                                        (1.5 * SCALE) if last else 1.5,
                                        mybir.AluOpType.mult, mybir.AluOpType.add)
                if last:
                    nc.vector.tensor_mul(rkT_sb[:, sib * 4:(sib + 1) * 4], yk, tk)
                else:
                    ykn = smal.tile([128, 4], f32, tag="kB", name="ykn")
                    nc.vector.tensor_mul(ykn, yk, tk)
                    yk = ykn
            # k RoPE -> kk2 (duplicated rows for the hh=1 score matmuls)
            t1k = tmpp.tile([64, 512], bf16, tag="t1")
            nc.vector.tensor_mul(t1k, s["kr"], cosk_sb[:, sp])
            ks = tmpp.tile([64, 512], bf16, tag="qs")
            nc.vector.tensor_copy(ks[0:32, :], s["kr"][32:64, :])
            nc.vector.tensor_copy(ks[32:64, :], s["kr"][0:32, :])
            t2k = tmpp.tile([64, 512], bf16, tag="t2")
            nc.vector.tensor_mul(t2k, ks, sink_sb[:, sp])
            nc.vector.tensor_add(kk2[0:64, sp], t1k, t2k)
            nc.vector.tensor_copy(kk2[64:128, sp], kk2[0:64, sp])
            # q RoPE + rq fold for p0 (rq broadcast via PE selector matmul)
            rope_q(s, sib, 0)

        def chains_b(sib):
            """p1 rsqrt+RoPE (hidden behind outproj of the previous block)."""
            s = st8[sib]
            s["rqt"][1] = newton_q(s, 1)
            rope_q(s, sib, 1)

        f32r = dt.float32r

        def dens(ps_att):
            den2 = smal.tile([2, 512], f32, tag="den2", bufs=2, name="den2")
            for hh in range(2):
                dh = smal.tile([1, 512], f32, tag="dh", bufs=4, name="dh")
                nc.scalar.copy(dh, ps_att[hh][64:65, :])
                nc.sync.dma_start(out=bass.AP(
                    tensor=den2.tensor, offset=den2[hh:hh + 1, :].offset,
                    ap=den2[hh:hh + 1, :].ap), in_=dh)
            return den2

        def den_recip(sib):
            """u=(1+e^-g)*den; s=1/u (DVE, runs early off the PE path)."""
            s = st8[sib]
            s["s_t"] = []
            for p in range(2):
                u_t = smal.tile([2, 512], f32, tag="den4", bufs=2, name="u")
                nc.vector.scalar_tensor_tensor(u_t, s["sig"][p], 1.0,
                                               s["den2"][p],
                                               mybir.AluOpType.add,
                                               mybir.AluOpType.mult)
                s_t = smal.tile([2, 512], f32, tag="st", bufs=2, name="s_t")
                nc.vector.reciprocal_approx_fast(out=s_t, in_=u_t)
                s["s_t"].append(s_t)

        def den_scale(s, p):
            """broadcast the per-token scale rows via PE."""
            ps_sb = psum.tile([128, 512], f32, tag="sc", bufs=2,
                              name="pssb")
            nc.tensor.matmul(ps_sb[:], eselb, s["s_t"][p], start=True, stop=True)
            sbc = bcp.tile([128, 512], f32, tag="sbc")
            nc.vector.tensor_copy(sbc, ps_sb[:])
            return sbc

        def attention_head(sib):
            """Interleaved p0/p1 J-loops: PE always has an independent
            scores/PV stream while ACT runs the exps."""
            s = st8[sib]
            B = sib
            ps_att = {}
            for p in range(2):
                for hh in range(2):
                    ps_att[(p, hh)] = psum.tile([128, 512], f32, tag="att",
                                                bufs=4, name=f"psatt{p}{hh}")
            for J in range(4 * B + 4):
                off = max(0, (J - 4 * B) * 128)
                ex = {}
                for p in range(2):
                    for hh in range(2):
                        rb = hh * 64
                        ps_s = psum.tile([128, 512], f32, tag="sc", bufs=2,
                                         name="pss")
                        nc.tensor.matmul(
                            ps_s[:, off:512],
                            kk2[rb:rb + 64, J * 128:(J + 1) * 128],
                            s["qf"][p][rb:rb + 64, off:512],
                            start=True, stop=True,
                            tile_position=(rb, 0))
                        et = expp.tile([128, 512], bf16, tag="expT", bufs=5,
                                       name="et")
                        nc.scalar.activation(et[:, off:512], ps_s[:, off:512],
                                             AF.Exp, scale=rkT_sb[:, J:J + 1])
                        if off > 0 or J == 4 * B:
                            nc.vector.tensor_mul(et[:, off:off + 128],
                                                 et[:, off:off + 128], tri)
                        ex[(p, hh)] = et
                for p in range(2):
                    for hh in range(2):
                        nc.tensor.matmul(
                            ps_att[(p, hh)][0:65, off:512],
                            v_sb[:, J, :],
                            ex[(p, hh)][:, off:512],
                            start=(J == 0), stop=(J == 4 * B + 3))

            s["ps_att"] = ps_att

        def attention_drain(sib):
            """Copy the four accumulators to SBUF + extract denominators."""
            s = st8[sib]
            ps_att = s["ps_att"]
            s["atr"] = [rawp.tile([128, 512], bf16, tag=f"atr{p}",
                                  name=f"atr{p}") for p in range(2)]
            s["den2"] = []
            for p in range(2):
                for hh in range(2):
                    rb = hh * 64
                    nc.vector.tensor_copy(s["atr"][p][rb:rb + 64, :],
                                          ps_att[(p, hh)][0:64, :])
                s["den2"].append(dens([ps_att[(p, 0)], ps_att[(p, 1)]]))
            s["at"] = [rawp.tile([128, 512], bf16, tag=f"at{p}",
                                 name=f"at{p}", bufs=2) for p in range(2)]

        def attention_tail(sib):
            s = st8[sib]
            for p in range(2):
                sbc = den_scale(s, p)
                for hh in range(2):
                    rb = hh * 64
                    nc.vector.tensor_mul(s["at"][p][rb:rb + 64, :],
                                         s["atr"][p][rb:rb + 64, :],
                                         sbc[rb:rb + 64, :])

        def outproj(sib):
            s = st8[sib]
            for ss in range(4 * sib, 4 * sib + 4):
                ls = (ss - 4 * sib) * 128
                ot = outs.tile([128, 2048], bf16, tag="ot")
                for qtr in range(4):
                    ps_o = psum.tile([128, 512], f32, tag="att", bufs=4,
                                     name="pso")
                    nc.tensor.matmul(ps_o[:], s["at"][0][:, ls:ls + 128],
                                     Wo_sb[:, 0, qtr * 512:(qtr + 1) * 512],
                                     start=True, stop=False)
                    nc.tensor.matmul(ps_o[:], s["at"][1][:, ls:ls + 128],
                                     Wo_sb[:, 1, qtr * 512:(qtr + 1) * 512],
                                     start=False, stop=True)
                    if qtr == 0:
                        nc.scalar.copy(ot[:, qtr * 512:(qtr + 1) * 512], ps_o[:])
                    else:
                        nc.vector.tensor_copy(ot[:, qtr * 512:(qtr + 1) * 512],
                                              ps_o[:])
                nc.gpsimd.dma_start(
                    out=out_d[ss * 128:(ss + 1) * 128, :],
                    in_=ot)

        # ---------------- pipelined schedule
        qkv_block(0)
        smalls(0)
        for B in range(NB):
            if B + 1 < NB:
                qkv_block(B + 1)
            chains_a(B)
            if B > 0:
                attention_drain(B - 1)
                den_recip(B - 1)
                attention_tail(B - 1)
                outproj(B - 1)
            chains_b(B)
            if B + 1 < NB:
                smalls(B + 1)
            attention_head(B)
        attention_drain(NB - 1)
        den_recip(NB - 1)
        attention_tail(NB - 1)
        outproj(NB - 1)

    nc.compile()
    return nc


def _get_nc():
    if "nc" not in _BUILT:
        _BUILT["nc"] = _build_nc()
    return _BUILT["nc"]


# ---------------------------------------------------------------- entry point
def _install_ntff_hook():
    import types
    try:
        import antenv
        if "antenv.axon_hooks" in sys.modules:
            return True
        mod = types.ModuleType("antenv.axon_hooks")
        holder = [None]
        mod.set_axon_ntff_profile_hook = lambda h: holder.__setitem__(0, h)
        mod.get_axon_ntff_profile_hook = lambda: holder[0]
        sys.modules["antenv.axon_hooks"] = mod
        antenv.axon_hooks = mod
        from trn_agent_boot.trn_boot import _ntff_profile_via_ctypes
        hook = _ntff_profile_via_ctypes("/opt/axon/libaxon_pjrt.so")
        if hook is None:
            return False
        mod.set_axon_ntff_profile_hook(hook)
        return True
    except Exception:
        return False


def kernel(hidden_states, Wq, Wk, Wv, Wo, g_q, g_k):
    global LAST_EXEC_NS
    from concourse.bass_utils import run_bass_kernel_spmd

    in_maps = _host_prep(hidden_states, Wq, Wk, Wv, Wo, g_q, g_k)
    nc = _get_nc()
    trace = os.environ.get("KERNEL_TRACE", "0") == "1"
    if trace:
        trace = _install_ntff_hook()
    res = run_bass_kernel_spmd(nc, in_maps, list(range(NCORES)), trace=trace)
    LAST_EXEC_NS = res.exec_time_ns
    out = np.zeros((S, H), np.float32)
    for c in range(NCORES):
        out += res.results[c]["out"].astype(np.float32)
    return out.reshape(1, S, H).astype(np.float32)
